# revision 28
# baseline (speedup 1.0000x reference)
"""GNN message-passing (R-GCN style) kernel for 8 Trainium2 NeuronCores.

Reference computation:
    msgs = einsum("eoi,ei->eo", W[widx], x[u])      # per-edge transform
    out  = relu(segment_sum(msgs, v, N))            # scatter-add + relu

Distribution strategy: edges are sharded by destination-node range
(12500 nodes per core), so each core owns a disjoint slice of the output
and no inter-core collective is needed.  W and x are replicated.

Device-side work (all FLOPs):
  Launch A: per-edge weight transform.  Edges are grouped by weight index
    (widx); each group's edges are packed 8-per-column and processed with
    a block-diagonal [128,128] @ [128,ncols] PE matmul (K = 8x16).  The
    block-diagonal weight operand lives in SBUF striped by j-slot
    (stripe j holds the weight bank contiguously at segment j), so it is
    built from the compact 1MB j-replicated bank W8 with 8 fully
    contiguous DMAs plus a zero memset split across three engines; the
    matmul lhsT reads it with a strided [128, 8, 16] access pattern.
    Group column ranges are variable (sized to actual per-group counts,
    maxed across cores so one SPMD program serves all 8 cores).  PSUM is
    accumulated in [128, 512] blocks drained by single large copies
    alternating between the vector and scalar engines.
  Launch B: segment-sum + ReLU.  Destination nodes are bucketed into
    128-node windows by descending degree, so each window is padded only
    to its own max degree (DN_k); windows with equal DN_k are batched
    into single vector tensor_reduce instructions (bf16 in/out engages
    the DVE 2x mode; sums of <=44 bf16 values keep absmax error well
    inside the 2e-2 budget).  ReLU on the scalar engine.

The host does data layout only: sharding, sorting/padding into the
static group structure, gathering x rows into the packed matmul operand,
and permuting the 16-float messages from widx-order to v-order between
the two launches.  (Device-side per-edge random access is not available:
the loadable GPSIMD ucode libraries are absent and indirect DMA has
32B/row descriptor granularity, far too slow for 200K rows/core.)
"""

import sys

sys.path.insert(0, "/opt/trn_rl_repo")

import numpy as np
import ml_dtypes

try:
    # bass_utils imports antenv.axon_hooks when tracing is requested via
    # env; some images lack that module — register a graceful stub so a
    # BASS_TRACE=1 environment degrades to "no trace" instead of crashing.
    import antenv.axon_hooks  # noqa: F401
except ImportError:
    import types

    import antenv

    _hooks = types.ModuleType("antenv.axon_hooks")
    _hooks._hook = None
    _hooks.set_axon_ntff_profile_hook = lambda h: setattr(_hooks, "_hook", h)
    _hooks.get_axon_ntff_profile_hook = lambda: _hooks._hook
    sys.modules["antenv.axon_hooks"] = _hooks
    antenv.axon_hooks = _hooks

import concourse.bacc as bacc
import concourse.mybir as mybir
import concourse.tile as tile
from concourse.bass_utils import run_bass_kernel_spmd

BF16 = ml_dtypes.bfloat16

# set by test harnesses: when True, launches run with trace=True and
# per-launch exec times land in LAST_EXEC_NS
TRACE = False
LAST_EXEC_NS = []

N_NODES = 100000
D = 16
NW = 256
N_CORES = 8
VSH = N_NODES // N_CORES          # 12500 destination nodes per core

CHUNK = 2048                      # A-side columns per DMA chunk
PSB = 512                         # A-side columns per PSUM block
NWIN = (VSH + 127) // 128         # 98 destination 128-node windows per core
B_MAX_FREE = 4096                 # B-side max elems/partition per sbuf tile
B_OUT_BF16 = True                 # B-side reduce/relu/output in bf16 (DVE 2x)


def _build_kernel_a(TCP, units):
    """units = list of (g, c0, c1) absolute column ranges, ascending, each
    within a single PSB-aligned block (and hence a single CHUNK)."""
    nc = bacc.Bacc(None, target_bir_lowering=False, debug=False)
    XU = nc.dram_tensor("XU", [128, TCP], mybir.dt.bfloat16, kind="ExternalInput")
    W8 = nc.dram_tensor("W8", [128, NW * D], mybir.dt.bfloat16, kind="ExternalInput")
    MSG = nc.dram_tensor("MSG", [128, TCP], mybir.dt.bfloat16, kind="ExternalOutput")

    # group units by psum block
    blocks = {}
    for g, c0, c1 in units:
        blocks.setdefault(c0 // PSB, []).append((g, c0, c1))

    with tile.TileContext(nc) as tc:
        with (
            tc.tile_pool(name="bd", bufs=1) as bdp,
            tc.tile_pool(name="xu", bufs=6) as xu_pool,
            tc.tile_pool(name="sbuf", bufs=3) as pool,
            tc.tile_pool(name="psum", bufs=2, space="PSUM") as psum_pool,
        ):
            # ---- build striped block-diagonal weight operand in SBUF ---
            # stripe j: partitions [16j,16j+16) hold the full bank at
            # segment j (cols [4096j, 4096j+4096)), zeros elsewhere.
            bd = bdp.tile([128, 8 * NW * D], mybir.dt.bfloat16, tag="bd")
            # memset split across engines (4096-aligned so each stripe
            # build-DMA depends on exactly one memset); uint32 views
            # halve the element count each engine has to write
            nc.vector.memset(bd[:, 0:4096].bitcast(mybir.dt.uint32), 0)
            nc.scalar.memzero(bd[:, 4096:8192])
            nc.gpsimd.memset(bd[:, 8192:8 * NW * D].bitcast(mybir.dt.uint32), 0)
            # stripe builds first on the fast sync queue (HWDGE): they
            # gate every matmul, so they go ahead of the XU stream;
            # ordered to chase the engines' expected memset finish times
            for j in (1, 0, 2, 3, 4, 5, 6, 7):
                nc.sync.dma_start(
                    out=bd[16 * j:16 * (j + 1), 4096 * j:4096 * (j + 1)],
                    in_=W8[16 * j:16 * (j + 1), :],
                )
            bdv = bd[:].rearrange("p (m g) -> p m g", g=NW)

            nchunks = TCP // CHUNK
            ncopy = 0
            for ch in range(nchunks):
                base = ch * CHUNK
                xu_t = xu_pool.tile([128, CHUNK], mybir.dt.bfloat16, tag="xu")
                nc.sync.dma_start(out=xu_t[:], in_=XU[:, base:base + CHUNK])
                out_t = pool.tile([128, CHUNK], mybir.dt.bfloat16, tag="out")
                for b in range(base // PSB, (base + CHUNK) // PSB):
                    us = blocks.get(b, [])
                    if not us:
                        continue
                    ps = psum_pool.tile([128, PSB], mybir.dt.float32, tag=f"ps{b % 4}")
                    for g, c0, c1 in us:
                        nc.tensor.matmul(
                            out=ps[:, c0 - b * PSB:c1 - b * PSB],
                            lhsT=bdv[:, :, g:g + 1],
                            rhs=xu_t[:, c0 - base:c1 - base],
                            start=True,
                            stop=True,
                        )
                    lo_ps = us[0][1] - b * PSB
                    hi_ps = us[-1][2] - b * PSB
                    lo = us[0][1] - base
                    hi = us[-1][2] - base
                    if ncopy % 2 == 0:
                        nc.vector.tensor_copy(out_t[:, lo:hi], ps[:, lo_ps:hi_ps])
                    else:
                        nc.scalar.copy(out=out_t[:, lo:hi], in_=ps[:, lo_ps:hi_ps])
                    ncopy += 1
                # half-chunk stores split across the gpsimd and sync
                # queues (sync's XU backlog drains early); subtile deps
                # let the first half ship while the second computes
                nc.gpsimd.dma_start(
                    out=MSG[:, base:base + CHUNK // 2],
                    in_=out_t[:, :CHUNK // 2],
                )
                nc.sync.dma_start(
                    out=MSG[:, base + CHUNK // 2:base + CHUNK],
                    in_=out_t[:, CHUNK // 2:],
                )
    nc.compile()
    return nc


def _build_kernel_b(PT, runs):
    """runs = list of (dn, k0, k1, poff) equal-DN window runs (chunked).
    MSGB is partition-major: MSGB[p, poff_k + o*dn_k + s] = slot s of
    component o of the node at rank k*128+p, so every window-run DMA is a
    fully contiguous 2D slice."""
    nc = bacc.Bacc(None, target_bir_lowering=False, debug=False)
    odt = mybir.dt.bfloat16 if B_OUT_BF16 else mybir.dt.float32
    MSGB = nc.dram_tensor("MSGB", [128, PT], mybir.dt.bfloat16, kind="ExternalInput")
    # partition-major output: OUTP[p, k*D+o] = out of node at rank k*128+p
    OUTP = nc.dram_tensor("OUTP", [128, NWIN * D], odt, kind="ExternalOutput")

    with nc.allow_low_precision("node sums of <=44 bf16 terms fit error budget"), \
            tile.TileContext(nc) as tc:
        with (
            tc.tile_pool(name="out", bufs=1) as outp,
            tc.tile_pool(name="sbuf", bufs=6) as pool,
        ):
            # single whole-launch output tile -> one contiguous final DMA
            out_t = outp.tile([128, NWIN * D], odt, tag="out")
            queues = [nc.sync, nc.gpsimd, nc.scalar]
            qbytes = [0, 0, 0]
            for ri, (dn, k0, k1, poff) in enumerate(runs):
                nw = k1 - k0
                msg_t = pool.tile([128, nw * D * dn], mybir.dt.bfloat16, tag="msg")
                qi = qbytes.index(min(qbytes))
                qbytes[qi] += nw * D * dn
                queues[qi].dma_start(
                    out=msg_t[:],
                    in_=MSGB[:, poff:poff + nw * D * dn],
                )
                # two pairwise-add halving levels (dn % 4 == 0) run in the
                # DVE 4x mode, then a short X-reduce of the quarters
                q = dn // 4
                h1 = pool.tile([128, nw * D * (dn // 2)], mybir.dt.bfloat16, tag="h1")
                v = msg_t[:].rearrange("p (w o s) -> p w o s", w=nw, o=D)
                # first halving level alternates vector/gpsimd to split
                # the elementwise load across engines
                l0_eng = nc.vector if ri % 2 == 0 else nc.gpsimd
                l0_eng.tensor_tensor(
                    out=h1[:], in0=v[:, :, :, :dn // 2], in1=v[:, :, :, dn // 2:],
                    op=mybir.AluOpType.add)
                h2 = pool.tile([128, nw * D * q], mybir.dt.bfloat16, tag="h2")
                v1 = h1[:].rearrange("p (w o s) -> p w o s", w=nw, o=D)
                nc.vector.tensor_tensor(
                    out=h2[:], in0=v1[:, :, :, :q], in1=v1[:, :, :, q:],
                    op=mybir.AluOpType.add)
                acc_t = pool.tile([128, nw * D], odt, tag="acc")
                nc.vector.tensor_reduce(
                    out=acc_t[:],
                    in_=h2[:].rearrange("p (w o s) -> p w o s", w=nw, o=D),
                    axis=mybir.AxisListType.X,
                    op=mybir.AluOpType.add,
                )
                nc.scalar.activation(
                    out_t[:, k0 * D:k1 * D], acc_t[:],
                    mybir.ActivationFunctionType.Relu)
            nc.scalar.dma_start(out=OUTP[:, :], in_=out_t[:])
    nc.compile()
    return nc


def _prep_a(u_s, widx_s, x_bf, colofs, TCP):
    """Pack one core's gathered x rows into the A-side matmul operand.

    Returns (XU [128, TCP] bf16, col(edge), j(edge)) where edge order is
    the stable widx sort of this core's edges.
    """
    ordA = np.argsort(widx_s, kind="stable")
    wA = widx_s[ordA]
    n = u_s.shape[0]
    cnts = np.bincount(wA, minlength=NW)
    starts = np.zeros(NW + 1, np.int64)
    np.cumsum(cnts, out=starts[1:])
    rank = np.arange(n) - starts[wA]
    col = colofs[wA] + rank // 8
    j = rank % 8

    xu3 = np.zeros((TCP * 8, D), BF16)
    xu3[col * 8 + j] = x_bf[u_s[ordA]]
    # [TCP, 8, 16] -> [8, 16, TCP] -> [128, TCP], row = 16j+i
    XU = np.ascontiguousarray(
        xu3.reshape(TCP, 8, D).transpose(1, 2, 0).reshape(128, TCP)
    )
    col_of_edge = np.empty(n, np.int64)
    col_of_edge[ordA] = col
    j_of_edge = np.empty(n, np.int64)
    j_of_edge[ordA] = j
    return XU, col_of_edge, j_of_edge


def kernel(x, W, u, v, widx):
    x = np.asarray(x, np.float32)
    W = np.asarray(W, np.float32)
    u = np.asarray(u).astype(np.int64)
    v = np.asarray(v).astype(np.int64)
    widx = np.asarray(widx).astype(np.int64)

    x_bf = x.astype(BF16)

    # compact j-replicated weight bank: W8[16j+i, 256o+g] = W[g, o, i]
    # (bank-transposed so a group's lhsT is a single-stride AP: the
    # striped SBUF operand bd[16j+i, 256*(16j+o)+g] reads as stride-256)
    WT = W.transpose(2, 1, 0)                          # [i, o, g]
    W8 = np.broadcast_to(WT[None], (8, D, D, NW))
    W8 = np.ascontiguousarray(W8.reshape(128, NW * D)).astype(BF16)

    # ---- shard by destination range -----------------------------------
    shard = v // VSH
    sel = [shard == m for m in range(N_CORES)]
    u_s = [u[s] for s in sel]
    v_s = [v[s] - m * VSH for m, s in enumerate(sel)]
    w_s = [widx[s] for s in sel]

    # ---- common A-side structure (max group size across cores) --------
    cnts = np.stack([np.bincount(ws, minlength=NW) for ws in w_s])   # [8, NW]
    NC = (cnts.max(axis=0) + 7) // 8                                 # cols per group
    NC = np.maximum(NC, 1)
    colofs = np.zeros(NW + 1, np.int64)
    np.cumsum(NC, out=colofs[1:])
    TC = int(colofs[-1])
    TCP = ((TC + CHUNK - 1) // CHUNK) * CHUNK

    units = []
    for g in range(NW):
        c = int(colofs[g])
        b = c + int(NC[g])
        while c < b:
            lim = min(b, (c // PSB + 1) * PSB)
            units.append((g, c, lim))
            c = lim

    # ---- common B-side structure (degree-sorted windows) --------------
    degs = np.stack([np.bincount(vs, minlength=VSH) for vs in v_s])  # [8, VSH]
    perms = [np.argsort(-degs[m], kind="stable") for m in range(N_CORES)]
    sdeg = np.stack([degs[m][perms[m]] for m in range(N_CORES)])     # desc
    DN = sdeg[:, ::128].max(axis=0).astype(np.int64)                 # [NWIN]
    DN = (np.maximum(DN, 1) + 3) // 4 * 4      # mult of 4 for add-halving
    poff = np.zeros(NWIN + 1, np.int64)
    np.cumsum(DN * D, out=poff[1:])
    PT = int(poff[-1])

    runs = []
    k = 0
    while k < NWIN:
        k2 = k
        while k2 < NWIN and DN[k2] == DN[k]:
            k2 += 1
        # chunk runs so each sbuf tile stays small
        dn = int(DN[k])
        max_nw = max(1, B_MAX_FREE // (D * dn))
        while k < k2:
            k1 = min(k2, k + max_nw)
            runs.append((dn, k, k1, int(poff[k])))
            k = k1

    # ---- host prep per core -------------------------------------------
    prepsA = [_prep_a(u_s[m], w_s[m], x_bf, colofs, TCP) for m in range(N_CORES)]

    # ---- launch A: per-edge transform ---------------------------------
    ncA = _build_kernel_a(TCP, units)
    in_maps_a = [{"XU": p[0], "W8": W8} for p in prepsA]
    LAST_EXEC_NS.clear()
    resA = run_bass_kernel_spmd(ncA, in_maps_a, list(range(N_CORES)), trace=TRACE)
    if TRACE:
        LAST_EXEC_NS.append(resA.exec_time_ns)

    # ---- host: permute messages widx-order -> v-order -----------------
    in_maps_b = []
    for m in range(N_CORES):
        msgsA = resA.results[m]["MSG"]                # [128, TCP] bf16
        _, col, j = prepsA[m]
        vecs = msgsA[(j * D)[:, None] + np.arange(D)[None, :], col[:, None]]

        vs = v_s[m]
        ordB = np.argsort(vs, kind="stable")
        vB = vs[ordB]
        deg = degs[m]
        startsB = np.zeros(VSH + 1, np.int64)
        np.cumsum(deg, out=startsB[1:])
        s_of = np.arange(vB.shape[0]) - startsB[vB]   # slot within node
        rank_of_node = np.empty(VSH, np.int64)
        rank_of_node[perms[m]] = np.arange(VSH)
        r = rank_of_node[vB]
        kw = r // 128
        p = r % 128
        dnk = DN[kw]
        base = poff[kw] + s_of
        flat = np.zeros((128, PT), BF16)
        flat[p[:, None], base[:, None] + np.arange(D)[None, :] * dnk[:, None]] = vecs[ordB]
        in_maps_b.append({"MSGB": flat})

    # ---- launch B: segment-sum + ReLU ---------------------------------
    ncB = _build_kernel_b(PT, runs)
    resB = run_bass_kernel_spmd(ncB, in_maps_b, list(range(N_CORES)), trace=TRACE)
    if TRACE:
        LAST_EXEC_NS.append(resB.exec_time_ns)

    out = np.empty((N_NODES, D), np.float32)
    for m in range(N_CORES):
        outP = resB.results[m]["OUTP"]                # [128, NWIN*D]
        byrank = outP.reshape(128, NWIN, D).transpose(1, 0, 2).reshape(NWIN * 128, D)
        out[m * VSH + perms[m]] = byrank[:VSH].astype(np.float32)
    return out


# revision 35
# speedup vs baseline: 1.2214x; 1.2214x over previous
"""GNN message-passing (R-GCN style) kernel for 8 Trainium2 NeuronCores.

Reference computation:
    msgs = einsum("eoi,ei->eo", W[widx], x[u])      # per-edge transform
    out  = relu(segment_sum(msgs, v, N))            # scatter-add + relu

Distribution strategy: edges are sharded by destination-node range
(12500 nodes per core), so each core owns a disjoint slice of the output
and no inter-core collective is needed.  W and x are replicated.

Device-side work (all FLOPs):
  Launch A: per-edge weight transform.  Weight groups are packed four to
    a matmul ("quads", paired by size so padding stays small): the
    [128,128] block-diagonal lhsT holds each quad member's 16x16 weight
    on two of the eight j-slots, and each rhs column carries 8 edges
    (2 per member group).  This quarters the stationary-weight traffic
    through the PE (the dominant tensor cost) versus one group per
    matmul, and shrinks the SBUF operand to 2.1MB so the zero-fill is
    cheap.  The operand layout bd[16j+i, 1024j + 64o + Q] makes every
    lhsT a single-stride access pattern AND every stripe build a fully
    contiguous [16, 1024] DMA from the 0.26MB host bank W8.  Quad column
    ranges are sized to the actual per-group edge counts (maxed across
    cores so one SPMD program serves all 8 cores).  Each quad owns one
    [128, 512] PSUM tile drained right after its matmul pieces by copies
    alternating between the vector and scalar engines.
  Launch B: segment-sum + ReLU.  Destination nodes are bucketed into
    128-node windows by descending degree, so each window is padded only
    to its own max degree DN_k (rounded to a multiple of 4); two
    pairwise bf16 add levels (DVE 2x mode) halve the slots twice, then a
    short X-reduce finishes each window batch.  The input is stored
    partition-major so every window-run load is one contiguous 2D DMA.
    ReLU on the scalar engine, one contiguous output store at the end.

The host does data layout only: sharding, sorting/padding into the
static structures, gathering x rows into the packed matmul operand, and
permuting the 16-float messages from widx-order to v-order between the
two launches.  (Device-side per-edge random access is not available:
the loadable GPSIMD ucode libraries are absent and indirect DMA has
32B/row descriptor granularity, far too slow for 200K rows/core.)
"""

import sys

sys.path.insert(0, "/opt/trn_rl_repo")

import numpy as np
import ml_dtypes

try:
    # bass_utils imports antenv.axon_hooks when tracing is requested via
    # env; some images lack that module — register a graceful stub so a
    # BASS_TRACE=1 environment degrades to "no trace" instead of crashing.
    import antenv.axon_hooks  # noqa: F401
except ImportError:
    import types

    import antenv

    _hooks = types.ModuleType("antenv.axon_hooks")
    _hooks._hook = None
    _hooks.set_axon_ntff_profile_hook = lambda h: setattr(_hooks, "_hook", h)
    _hooks.get_axon_ntff_profile_hook = lambda: _hooks._hook
    sys.modules["antenv.axon_hooks"] = _hooks
    antenv.axon_hooks = _hooks

import concourse.bacc as bacc
import concourse.mybir as mybir
import concourse.tile as tile
from concourse.bass_utils import run_bass_kernel_spmd

BF16 = ml_dtypes.bfloat16

# set by test harnesses: when True, launches run with trace=True and
# per-launch exec times land in LAST_EXEC_NS
TRACE = False
LAST_EXEC_NS = []

N_NODES = 100000
D = 16
NW = 256
NQ = NW // 4                      # weight quads (4 groups per matmul)
N_CORES = 8
VSH = N_NODES // N_CORES          # 12500 destination nodes per core

CHUNK = 2048                      # A-side columns per DMA chunk
PSB = 512                         # A-side columns per PSUM tile
NWIN = (VSH + 127) // 128         # 98 destination 128-node windows per core
B_MAX_FREE = 4096                 # B-side max elems/partition per sbuf tile
B_OUT_BF16 = True                 # B-side reduce/relu/output in bf16 (DVE 2x)


def _build_kernel_a(TCP, pieces):
    """pieces = list of (Q, reg, c0, c1) ascending in c0: matmul piece of
    quad Q covering absolute cols [c0, c1), accumulated in PSUM tile
    (Q, reg) and drained immediately after."""
    nc = bacc.Bacc(None, target_bir_lowering=False, debug=False)
    XU = nc.dram_tensor("XU", [128, TCP], mybir.dt.bfloat16, kind="ExternalInput")
    W8 = nc.dram_tensor("W8", [128, D * NQ], mybir.dt.bfloat16, kind="ExternalInput")
    MSG = nc.dram_tensor("MSG", [128, TCP], mybir.dt.bfloat16, kind="ExternalOutput")

    by_chunk = {}
    for pc in pieces:
        by_chunk.setdefault(pc[2] // CHUNK, []).append(pc)

    with tile.TileContext(nc) as tc:
        with (
            tc.tile_pool(name="bd", bufs=1) as bdp,
            tc.tile_pool(name="xu", bufs=6) as xu_pool,
            tc.tile_pool(name="sbuf", bufs=3) as pool,
            tc.tile_pool(name="psum", bufs=2, space="PSUM") as psum_pool,
        ):
            # ---- striped quad weight operand in SBUF -------------------
            # bd[16j+i, 1024j + 64o + Q] = W[quad Q member j//2][o, i];
            # stripe j is the contiguous block [16j:16j+16, 1024j:1024j+1024]
            bd = bdp.tile([128, 8 * D * NQ], mybir.dt.bfloat16, tag="bd")
            nc.vector.memset(bd[:, 0:4096].bitcast(mybir.dt.uint32), 0)
            nc.gpsimd.memset(bd[:, 4096:8192].bitcast(mybir.dt.uint32), 0)
            for j in range(8):
                nc.sync.dma_start(
                    out=bd[16 * j:16 * (j + 1), 1024 * j:1024 * (j + 1)],
                    in_=W8[16 * j:16 * (j + 1), :],
                )
            bdv = bd[:].rearrange("p (m q) -> p m q", q=NQ)

            nchunks = TCP // CHUNK
            ncopy = 0
            ps_tiles = {}
            for ch in range(nchunks):
                base = ch * CHUNK
                xu_t = xu_pool.tile([128, CHUNK], mybir.dt.bfloat16, tag="xu")
                nc.sync.dma_start(out=xu_t[:], in_=XU[:, base:base + CHUNK])
                out_t = pool.tile([128, CHUNK], mybir.dt.bfloat16, tag="out")
                for Q, reg, c0, c1 in by_chunk.get(ch, []):
                    key = (Q, reg)
                    if key not in ps_tiles:
                        ps_new = psum_pool.tile([128, PSB], mybir.dt.float32,
                                                tag=f"ps{len(ps_tiles) % 4}")
                        ps_tiles[key] = (ps_new, c0)  # c0 = tile col origin
                    ps, orig = ps_tiles[key]
                    nc.tensor.matmul(
                        out=ps[:, c0 - orig:c1 - orig],
                        lhsT=bdv[:, :, Q:Q + 1],
                        rhs=xu_t[:, c0 - base:c1 - base],
                        start=True,
                        stop=True,
                    )
                    if ncopy % 2 == 0:
                        nc.vector.tensor_copy(
                            out_t[:, c0 - base:c1 - base], ps[:, c0 - orig:c1 - orig])
                    else:
                        nc.scalar.copy(
                            out=out_t[:, c0 - base:c1 - base],
                            in_=ps[:, c0 - orig:c1 - orig])
                    ncopy += 1
                # half-chunk stores on the gpsimd queue (keeps sync free
                # for the XU stream and scalar free for drains)
                nc.gpsimd.dma_start(
                    out=MSG[:, base:base + CHUNK // 2],
                    in_=out_t[:, :CHUNK // 2],
                )
                nc.gpsimd.dma_start(
                    out=MSG[:, base + CHUNK // 2:base + CHUNK],
                    in_=out_t[:, CHUNK // 2:],
                )
    nc.compile()
    return nc


def _build_kernel_b(PT, runs):
    """runs = list of (dn, k0, k1, poff) equal-DN window runs (chunked).
    MSGB is partition-major: MSGB[p, poff_k + o*dn_k + s] = slot s of
    component o of the node at rank k*128+p, so every window-run DMA is a
    fully contiguous 2D slice."""
    nc = bacc.Bacc(None, target_bir_lowering=False, debug=False)
    odt = mybir.dt.bfloat16 if B_OUT_BF16 else mybir.dt.float32
    MSGB = nc.dram_tensor("MSGB", [128, PT], mybir.dt.bfloat16, kind="ExternalInput")
    # partition-major output: OUTP[p, k*D+o] = out of node at rank k*128+p
    OUTP = nc.dram_tensor("OUTP", [128, NWIN * D], odt, kind="ExternalOutput")

    with nc.allow_low_precision("node sums of <=44 bf16 terms fit error budget"), \
            tile.TileContext(nc) as tc:
        with (
            tc.tile_pool(name="out", bufs=1) as outp,
            tc.tile_pool(name="sbuf", bufs=6) as pool,
        ):
            # single whole-launch output tile -> one contiguous final DMA
            out_t = outp.tile([128, NWIN * D], odt, tag="out")
            queues = [nc.sync, nc.gpsimd, nc.scalar]
            qbytes = [0, 0, 0]
            for ri, (dn, k0, k1, poff) in enumerate(runs):
                nw = k1 - k0
                msg_t = pool.tile([128, nw * D * dn], mybir.dt.bfloat16, tag="msg")
                qi = qbytes.index(min(qbytes))
                qbytes[qi] += nw * D * dn
                queues[qi].dma_start(
                    out=msg_t[:],
                    in_=MSGB[:, poff:poff + nw * D * dn],
                )
                # two pairwise-add halving levels (dn % 4 == 0) in the
                # DVE 2x mode, then a short X-reduce of the quarters
                q = dn // 4
                h1 = pool.tile([128, nw * D * (dn // 2)], mybir.dt.bfloat16, tag="h1")
                v = msg_t[:].rearrange("p (w o s) -> p w o s", w=nw, o=D)
                nc.vector.tensor_tensor(
                    out=h1[:], in0=v[:, :, :, :dn // 2], in1=v[:, :, :, dn // 2:],
                    op=mybir.AluOpType.add)
                h2 = pool.tile([128, nw * D * q], mybir.dt.bfloat16, tag="h2")
                v1 = h1[:].rearrange("p (w o s) -> p w o s", w=nw, o=D)
                nc.vector.tensor_tensor(
                    out=h2[:], in0=v1[:, :, :, :q], in1=v1[:, :, :, q:],
                    op=mybir.AluOpType.add)
                acc_t = pool.tile([128, nw * D], odt, tag="acc")
                nc.vector.tensor_reduce(
                    out=acc_t[:],
                    in_=h2[:].rearrange("p (w o s) -> p w o s", w=nw, o=D),
                    axis=mybir.AxisListType.X,
                    op=mybir.AluOpType.add,
                )
                nc.scalar.activation(
                    out_t[:, k0 * D:k1 * D], acc_t[:],
                    mybir.ActivationFunctionType.Relu)
            nc.scalar.dma_start(out=OUTP[:, :], in_=out_t[:])
    nc.compile()
    return nc


def _prep_a(u_s, widx_s, x_bf, qof, tof, qcolofs, TCP):
    """Pack one core's gathered x rows into the A-side matmul operand.

    Returns (XU [128, TCP] bf16, col(edge), j(edge)) where edge order is
    the stable widx sort of this core's edges.
    """
    ordA = np.argsort(widx_s, kind="stable")
    wA = widx_s[ordA]
    n = u_s.shape[0]
    cnts = np.bincount(wA, minlength=NW)
    starts = np.zeros(NW + 1, np.int64)
    np.cumsum(cnts, out=starts[1:])
    rank = np.arange(n) - starts[wA]
    col = qcolofs[qof[wA]] + rank // 2
    j = 2 * tof[wA] + rank % 2

    xu3 = np.zeros((TCP * 8, D), BF16)
    xu3[col * 8 + j] = x_bf[u_s[ordA]]
    # [TCP, 8, 16] -> [8, 16, TCP] -> [128, TCP], row = 16j+i
    XU = np.ascontiguousarray(
        xu3.reshape(TCP, 8, D).transpose(1, 2, 0).reshape(128, TCP)
    )
    col_of_edge = np.empty(n, np.int64)
    col_of_edge[ordA] = col
    j_of_edge = np.empty(n, np.int64)
    j_of_edge[ordA] = j
    return XU, col_of_edge, j_of_edge


def prep_all(x, W, u, v, widx):
    """Host-side layout shared by kernel() and benchmarks."""
    x = np.asarray(x, np.float32)
    W = np.asarray(W, np.float32)
    u = np.asarray(u).astype(np.int64)
    v = np.asarray(v).astype(np.int64)
    widx = np.asarray(widx).astype(np.int64)
    x_bf = x.astype(BF16)

    # ---- shard by destination range -----------------------------------
    shard = v // VSH
    sel = [shard == m for m in range(N_CORES)]
    u_s = [u[s] for s in sel]
    v_s = [v[s] - m * VSH for m, s in enumerate(sel)]
    w_s = [widx[s] for s in sel]

    # ---- A-side quad structure (common across cores) ------------------
    cnts = np.stack([np.bincount(ws, minlength=NW) for ws in w_s])
    nc2 = np.maximum((cnts.max(axis=0) + 1) // 2, 1)   # cols per group
    order = np.argsort(nc2, kind="stable")
    qg = order.reshape(NQ, 4)                          # quad -> 4 groups
    qof = np.empty(NW, np.int64)
    tof = np.empty(NW, np.int64)
    for Q in range(NQ):
        for t in range(4):
            qof[qg[Q, t]] = Q
            tof[qg[Q, t]] = t
    qcols = nc2[qg].max(axis=1)
    qcolofs = np.zeros(NQ + 1, np.int64)
    np.cumsum(qcols, out=qcolofs[1:])
    TC = int(qcolofs[-1])
    TCP = ((TC + CHUNK - 1) // CHUNK) * CHUNK

    pieces = []
    for Q in range(NQ):
        qs = int(qcolofs[Q])
        qe = qs + int(qcols[Q])
        c = qs
        while c < qe:
            lim = min(qe, (c // CHUNK + 1) * CHUNK, qs + ((c - qs) // PSB + 1) * PSB)
            pieces.append((Q, (c - qs) // PSB, c, lim))
            c = lim
    pieces.sort(key=lambda pc: pc[2])

    # quad weight bank: W8[16j+i, 64o + Q] = W[qg[Q, j//2], o, i]
    A = W[qg]                                          # [NQ, 4, D, D] (Q,t,o,i)
    arr = A.transpose(1, 3, 2, 0)                      # [t, i, o, Q]
    W8 = np.ascontiguousarray(
        np.repeat(arr, 2, axis=0).reshape(128, D * NQ)).astype(BF16)

    # ---- B-side degree-sorted window structure ------------------------
    degs = np.stack([np.bincount(vs, minlength=VSH) for vs in v_s])
    perms = [np.argsort(-degs[m], kind="stable") for m in range(N_CORES)]
    sdeg = np.stack([degs[m][perms[m]] for m in range(N_CORES)])
    DN = sdeg[:, ::128].max(axis=0).astype(np.int64)
    DN = (np.maximum(DN, 1) + 3) // 4 * 4      # mult of 4 for add-halving
    poff = np.zeros(NWIN + 1, np.int64)
    np.cumsum(DN * D, out=poff[1:])
    PT = int(poff[-1])

    runs = []
    k = 0
    while k < NWIN:
        k2 = k
        while k2 < NWIN and DN[k2] == DN[k]:
            k2 += 1
        dn = int(DN[k])
        max_nw = max(1, B_MAX_FREE // (D * dn))
        while k < k2:
            k1 = min(k2, k + max_nw)
            runs.append((dn, k, k1, int(poff[k])))
            k = k1

    prepsA = [_prep_a(u_s[m], w_s[m], x_bf, qof, tof, qcolofs, TCP)
              for m in range(N_CORES)]
    return dict(x_bf=x_bf, u_s=u_s, v_s=v_s, w_s=w_s, W8=W8, TCP=TCP,
                pieces=pieces, degs=degs, perms=perms, DN=DN, poff=poff,
                PT=PT, runs=runs, prepsA=prepsA)


def make_in_maps_b(P, resA):
    """Permute launch A's messages into the B-side window layout."""
    in_maps_b = []
    for m in range(N_CORES):
        msgsA = resA.results[m]["MSG"]                # [128, TCP] bf16
        _, col, j = P["prepsA"][m]
        vecs = msgsA[(j * D)[:, None] + np.arange(D)[None, :], col[:, None]]

        vs = P["v_s"][m]
        ordB = np.argsort(vs, kind="stable")
        vB = vs[ordB]
        deg = P["degs"][m]
        startsB = np.zeros(VSH + 1, np.int64)
        np.cumsum(deg, out=startsB[1:])
        s_of = np.arange(vB.shape[0]) - startsB[vB]   # slot within node
        rank_of_node = np.empty(VSH, np.int64)
        rank_of_node[P["perms"][m]] = np.arange(VSH)
        r = rank_of_node[vB]
        kw = r // 128
        p = r % 128
        dnk = P["DN"][kw]
        base = P["poff"][kw] + s_of
        flat = np.zeros((128, P["PT"]), BF16)
        flat[p[:, None], base[:, None] + np.arange(D)[None, :] * dnk[:, None]] = vecs[ordB]
        in_maps_b.append({"MSGB": flat})
    return in_maps_b


def kernel(x, W, u, v, widx):
    P = prep_all(x, W, u, v, widx)

    # ---- launch A: per-edge transform ---------------------------------
    ncA = _build_kernel_a(P["TCP"], P["pieces"])
    in_maps_a = [{"XU": p[0], "W8": P["W8"]} for p in P["prepsA"]]
    LAST_EXEC_NS.clear()
    resA = run_bass_kernel_spmd(ncA, in_maps_a, list(range(N_CORES)), trace=TRACE)
    if TRACE:
        LAST_EXEC_NS.append(resA.exec_time_ns)

    # ---- host: permute messages widx-order -> v-order -----------------
    in_maps_b = make_in_maps_b(P, resA)

    # ---- launch B: segment-sum + ReLU ---------------------------------
    ncB = _build_kernel_b(P["PT"], P["runs"])
    resB = run_bass_kernel_spmd(ncB, in_maps_b, list(range(N_CORES)), trace=TRACE)
    if TRACE:
        LAST_EXEC_NS.append(resB.exec_time_ns)

    out = np.empty((N_NODES, D), np.float32)
    for m in range(N_CORES):
        outP = resB.results[m]["OUTP"]                # [128, NWIN*D]
        byrank = outP.reshape(128, NWIN, D).transpose(1, 0, 2).reshape(NWIN * 128, D)
        out[m * VSH + P["perms"][m]] = byrank[:VSH].astype(np.float32)
    return out


# revision 39
# speedup vs baseline: 1.3368x; 1.0945x over previous
"""GNN message-passing (R-GCN style) kernel for 8 Trainium2 NeuronCores.

Reference computation:
    msgs = einsum("eoi,ei->eo", W[widx], x[u])      # per-edge transform
    out  = relu(segment_sum(msgs, v, N))            # scatter-add + relu

Distribution strategy: edges are sharded by destination-node range
(12500 nodes per core), so each core owns a disjoint slice of the output
and no inter-core collective is needed.  W and x are replicated.

Device-side work (all FLOPs):
  Launch A: per-edge weight transform.  Weight groups are packed four to
    a matmul ("quads", paired by size so padding stays small): the
    [128,128] block-diagonal lhsT holds each quad member's 16x16 weight
    on two of the eight j-slots, and each rhs column carries 8 edges
    (2 per member group).  This quarters the stationary-weight traffic
    through the PE (the dominant tensor cost) versus one group per
    matmul, and shrinks the SBUF operand to 2.1MB so the zero-fill is
    cheap.  The operand layout bd[16j+i, 1024j + 64o + Q] makes every
    lhsT a single-stride access pattern AND every stripe build a fully
    contiguous [16, 1024] DMA from the 0.26MB host bank W8.  Quad column
    ranges are sized to the actual per-group edge counts (maxed across
    cores so one SPMD program serves all 8 cores).  Each quad owns one
    [128, 512] PSUM tile drained right after its matmul pieces by copies
    alternating between the vector and scalar engines.
  Launch B: segment-sum + ReLU.  Destination nodes are bucketed into
    128-node windows by descending degree, so each window is padded only
    to its own max degree DN_k (rounded to a multiple of 4); two
    pairwise bf16 add levels (DVE 2x mode) halve the slots twice, then a
    short X-reduce finishes each window batch.  The input is stored
    partition-major so every window-run load is one contiguous 2D DMA.
    ReLU on the scalar engine, one contiguous output store at the end.

The host does data layout only: sharding, sorting/padding into the
static structures, gathering x rows into the packed matmul operand, and
permuting the 16-float messages from widx-order to v-order between the
two launches.  (Device-side per-edge random access is not available:
the loadable GPSIMD ucode libraries are absent and indirect DMA has
32B/row descriptor granularity, far too slow for 200K rows/core.)
"""

import sys

sys.path.insert(0, "/opt/trn_rl_repo")

import numpy as np
import ml_dtypes

try:
    # bass_utils imports antenv.axon_hooks when tracing is requested via
    # env; some images lack that module — register a graceful stub so a
    # BASS_TRACE=1 environment degrades to "no trace" instead of crashing.
    import antenv.axon_hooks  # noqa: F401
except ImportError:
    import types

    import antenv

    _hooks = types.ModuleType("antenv.axon_hooks")
    _hooks._hook = None
    _hooks.set_axon_ntff_profile_hook = lambda h: setattr(_hooks, "_hook", h)
    _hooks.get_axon_ntff_profile_hook = lambda: _hooks._hook
    sys.modules["antenv.axon_hooks"] = _hooks
    antenv.axon_hooks = _hooks

import concourse.bacc as bacc
import concourse.mybir as mybir
import concourse.tile as tile
from concourse.bass_utils import run_bass_kernel_spmd

BF16 = ml_dtypes.bfloat16

# set by test harnesses: when True, launches run with trace=True and
# per-launch exec times land in LAST_EXEC_NS
TRACE = False
LAST_EXEC_NS = []

N_NODES = 100000
D = 16
NW = 256
NQ = NW // 4                      # weight quads (4 groups per matmul)
N_CORES = 8
VSH = N_NODES // N_CORES          # 12500 destination nodes per core

CHUNK = 2048                      # A-side columns per DMA chunk
PSB = 512                         # A-side columns per PSUM tile
NWIN = (VSH + 127) // 128         # 98 destination 128-node windows per core
B_MAX_FREE = 4096                 # B-side max elems/partition per sbuf tile
B_OUT_BF16 = True                 # B-side reduce/relu/output in bf16 (DVE 2x)


def _build_kernel_a(TCP, pieces):
    """pieces = list of (Q, reg, c0, c1) ascending in c0: matmul piece of
    quad Q covering absolute cols [c0, c1), accumulated in PSUM tile
    (Q, reg) and drained immediately after."""
    nc = bacc.Bacc(None, target_bir_lowering=False, debug=False)
    XU = nc.dram_tensor("XU", [128, TCP], mybir.dt.bfloat16, kind="ExternalInput")
    W8 = nc.dram_tensor("W8", [128, D * NQ], mybir.dt.bfloat16, kind="ExternalInput")
    MSG = nc.dram_tensor("MSG", [128, TCP], mybir.dt.bfloat16, kind="ExternalOutput")

    by_chunk = {}
    for pc in pieces:
        by_chunk.setdefault(pc[2] // CHUNK, []).append(pc)

    with tile.TileContext(nc) as tc:
        with (
            tc.tile_pool(name="bd", bufs=1) as bdp,
            tc.tile_pool(name="xu", bufs=6) as xu_pool,
            tc.tile_pool(name="sbuf", bufs=5) as pool,
            tc.tile_pool(name="psum", bufs=2, space="PSUM") as psum_pool,
        ):
            # ---- striped quad weight operand in SBUF -------------------
            # bd[16j+i, 1024j + 64o + Q] = W[quad Q member j//2][o, i];
            # stripe j is the contiguous block [16j:16j+16, 1024j:1024j+1024]
            bd = bdp.tile([128, 8 * D * NQ], mybir.dt.bfloat16, tag="bd")
            nc.vector.memset(bd[:, 0:4096].bitcast(mybir.dt.uint32), 0)
            nc.gpsimd.memset(bd[:, 4096:8192].bitcast(mybir.dt.uint32), 0)
            bdv = bd[:].rearrange("p (m q) -> p m q", q=NQ)

            nchunks = TCP // CHUNK
            ncopy = 0
            ps_tiles = {}
            xu_tiles = {}
            # first XU chunk ahead of the stripe builds so it lands while
            # the memsets run; contiguous stripe builds follow on the
            # sync queue; remaining chunks stream after
            for ch in range(min(1, nchunks)):
                xu_t = xu_pool.tile([128, CHUNK], mybir.dt.bfloat16, tag="xu")
                nc.sync.dma_start(out=xu_t[:], in_=XU[:, ch * CHUNK:(ch + 1) * CHUNK])
                xu_tiles[ch] = xu_t
            for j in range(8):
                nc.sync.dma_start(
                    out=bd[16 * j:16 * (j + 1), 1024 * j:1024 * (j + 1)],
                    in_=W8[16 * j:16 * (j + 1), :],
                )
            for ch in range(nchunks):
                base = ch * CHUNK
                if ch in xu_tiles:
                    xu_t = xu_tiles[ch]
                else:
                    xu_t = xu_pool.tile([128, CHUNK], mybir.dt.bfloat16, tag="xu")
                    nc.sync.dma_start(out=xu_t[:], in_=XU[:, base:base + CHUNK])
                out_t = pool.tile([128, CHUNK], mybir.dt.bfloat16, tag="out")
                for Q, reg, c0, c1 in by_chunk.get(ch, []):
                    key = (Q, reg)
                    if key not in ps_tiles:
                        ps_new = psum_pool.tile([128, PSB], mybir.dt.float32,
                                                tag=f"ps{len(ps_tiles) % 4}")
                        ps_tiles[key] = (ps_new, c0)  # c0 = tile col origin
                    ps, orig = ps_tiles[key]
                    nc.tensor.matmul(
                        out=ps[:, c0 - orig:c1 - orig],
                        lhsT=bdv[:, :, Q:Q + 1],
                        rhs=xu_t[:, c0 - base:c1 - base],
                        start=True,
                        stop=True,
                    )
                    if ncopy % 2 == 0:
                        nc.vector.tensor_copy(
                            out_t[:, c0 - base:c1 - base], ps[:, c0 - orig:c1 - orig])
                    else:
                        nc.scalar.copy(
                            out=out_t[:, c0 - base:c1 - base],
                            in_=ps[:, c0 - orig:c1 - orig])
                    ncopy += 1
                # half-chunk stores on the gpsimd queue (keeps sync free
                # for the XU stream and scalar free for drains)
                nc.gpsimd.dma_start(
                    out=MSG[:, base:base + CHUNK // 2],
                    in_=out_t[:, :CHUNK // 2],
                )
                nc.gpsimd.dma_start(
                    out=MSG[:, base + CHUNK // 2:base + CHUNK],
                    in_=out_t[:, CHUNK // 2:],
                )
    nc.compile()
    return nc


def _build_kernel_b(PT, runs):
    """runs = list of (dn, k0, k1, poff) equal-DN window runs (chunked).
    MSGB is partition-major: MSGB[p, poff_k + o*dn_k + s] = slot s of
    component o of the node at rank k*128+p, so every window-run DMA is a
    fully contiguous 2D slice."""
    nc = bacc.Bacc(None, target_bir_lowering=False, debug=False)
    odt = mybir.dt.bfloat16 if B_OUT_BF16 else mybir.dt.float32
    MSGB = nc.dram_tensor("MSGB", [128, PT], mybir.dt.bfloat16, kind="ExternalInput")
    # partition-major output: OUTP[p, k*D+o] = out of node at rank k*128+p
    OUTP = nc.dram_tensor("OUTP", [128, NWIN * D], odt, kind="ExternalOutput")

    with nc.allow_low_precision("node sums of <=44 bf16 terms fit error budget"), \
            tile.TileContext(nc) as tc:
        with (
            tc.tile_pool(name="out", bufs=1) as outp,
            tc.tile_pool(name="sbuf", bufs=6) as pool,
        ):
            # single whole-launch output tile -> one contiguous final DMA
            out_t = outp.tile([128, NWIN * D], odt, tag="out")
            queues = [nc.sync, nc.gpsimd, nc.scalar]
            qbytes = [0, 0, 0]
            for ri, (dn, k0, k1, poff) in enumerate(runs):
                nw = k1 - k0
                msg_t = pool.tile([128, nw * D * dn], mybir.dt.bfloat16, tag="msg")
                # weight gpsimd (SWDGE, slower) at half the HWDGE queues
                w = [1.0, 2.0, 1.0]
                qi = min(range(3), key=lambda i: qbytes[i] * w[i])
                qbytes[qi] += nw * D * dn
                queues[qi].dma_start(
                    out=msg_t[:],
                    in_=MSGB[:, poff:poff + nw * D * dn],
                )
                # two pairwise-add halving levels (dn % 4 == 0) in the
                # DVE 2x mode, then a short X-reduce of the quarters
                q = dn // 4
                h1 = pool.tile([128, nw * D * (dn // 2)], mybir.dt.bfloat16, tag="h1")
                v = msg_t[:].rearrange("p (w o s) -> p w o s", w=nw, o=D)
                nc.vector.tensor_tensor(
                    out=h1[:], in0=v[:, :, :, :dn // 2], in1=v[:, :, :, dn // 2:],
                    op=mybir.AluOpType.add)
                h2 = pool.tile([128, nw * D * q], mybir.dt.bfloat16, tag="h2")
                v1 = h1[:].rearrange("p (w o s) -> p w o s", w=nw, o=D)
                nc.vector.tensor_tensor(
                    out=h2[:], in0=v1[:, :, :, :q], in1=v1[:, :, :, q:],
                    op=mybir.AluOpType.add)
                acc_t = pool.tile([128, nw * D], odt, tag="acc")
                nc.vector.tensor_reduce(
                    out=acc_t[:],
                    in_=h2[:].rearrange("p (w o s) -> p w o s", w=nw, o=D),
                    axis=mybir.AxisListType.X,
                    op=mybir.AluOpType.add,
                )
                nc.scalar.activation(
                    out_t[:, k0 * D:k1 * D], acc_t[:],
                    mybir.ActivationFunctionType.Relu)
            third = (NWIN // 3) * D
            nc.scalar.dma_start(out=OUTP[:, :third], in_=out_t[:, :third])
            nc.scalar.dma_start(out=OUTP[:, third:2 * third], in_=out_t[:, third:2 * third])
            nc.scalar.dma_start(out=OUTP[:, 2 * third:], in_=out_t[:, 2 * third:])
    nc.compile()
    return nc


def _prep_a(u_s, widx_s, x_bf, qof, tof, qcolofs, TCP):
    """Pack one core's gathered x rows into the A-side matmul operand.

    Returns (XU [128, TCP] bf16, col(edge), j(edge)) where edge order is
    the stable widx sort of this core's edges.
    """
    ordA = np.argsort(widx_s, kind="stable")
    wA = widx_s[ordA]
    n = u_s.shape[0]
    cnts = np.bincount(wA, minlength=NW)
    starts = np.zeros(NW + 1, np.int64)
    np.cumsum(cnts, out=starts[1:])
    rank = np.arange(n) - starts[wA]
    col = qcolofs[qof[wA]] + rank // 2
    j = 2 * tof[wA] + rank % 2

    xu3 = np.zeros((TCP * 8, D), BF16)
    xu3[col * 8 + j] = x_bf[u_s[ordA]]
    # [TCP, 8, 16] -> [8, 16, TCP] -> [128, TCP], row = 16j+i
    XU = np.ascontiguousarray(
        xu3.reshape(TCP, 8, D).transpose(1, 2, 0).reshape(128, TCP)
    )
    col_of_edge = np.empty(n, np.int64)
    col_of_edge[ordA] = col
    j_of_edge = np.empty(n, np.int64)
    j_of_edge[ordA] = j
    return XU, col_of_edge, j_of_edge


def prep_all(x, W, u, v, widx):
    """Host-side layout shared by kernel() and benchmarks."""
    x = np.asarray(x, np.float32)
    W = np.asarray(W, np.float32)
    u = np.asarray(u).astype(np.int64)
    v = np.asarray(v).astype(np.int64)
    widx = np.asarray(widx).astype(np.int64)
    x_bf = x.astype(BF16)

    # ---- shard by destination range -----------------------------------
    shard = v // VSH
    sel = [shard == m for m in range(N_CORES)]
    u_s = [u[s] for s in sel]
    v_s = [v[s] - m * VSH for m, s in enumerate(sel)]
    w_s = [widx[s] for s in sel]

    # ---- A-side quad structure (common across cores) ------------------
    cnts = np.stack([np.bincount(ws, minlength=NW) for ws in w_s])
    nc2 = np.maximum((cnts.max(axis=0) + 1) // 2, 1)   # cols per group
    order = np.argsort(nc2, kind="stable")
    qg = order.reshape(NQ, 4)                          # quad -> 4 groups
    qof = np.empty(NW, np.int64)
    tof = np.empty(NW, np.int64)
    for Q in range(NQ):
        for t in range(4):
            qof[qg[Q, t]] = Q
            tof[qg[Q, t]] = t
    qcols = nc2[qg].max(axis=1)
    qcolofs = np.zeros(NQ + 1, np.int64)
    np.cumsum(qcols, out=qcolofs[1:])
    TC = int(qcolofs[-1])
    TCP = ((TC + CHUNK - 1) // CHUNK) * CHUNK

    pieces = []
    for Q in range(NQ):
        qs = int(qcolofs[Q])
        qe = qs + int(qcols[Q])
        c = qs
        while c < qe:
            lim = min(qe, (c // CHUNK + 1) * CHUNK, qs + ((c - qs) // PSB + 1) * PSB)
            pieces.append((Q, (c - qs) // PSB, c, lim))
            c = lim
    pieces.sort(key=lambda pc: pc[2])

    # quad weight bank: W8[16j+i, 64o + Q] = W[qg[Q, j//2], o, i]
    A = W[qg]                                          # [NQ, 4, D, D] (Q,t,o,i)
    arr = A.transpose(1, 3, 2, 0)                      # [t, i, o, Q]
    W8 = np.ascontiguousarray(
        np.repeat(arr, 2, axis=0).reshape(128, D * NQ)).astype(BF16)

    # ---- B-side degree-sorted window structure ------------------------
    degs = np.stack([np.bincount(vs, minlength=VSH) for vs in v_s])
    perms = [np.argsort(-degs[m], kind="stable") for m in range(N_CORES)]
    sdeg = np.stack([degs[m][perms[m]] for m in range(N_CORES)])
    DN = sdeg[:, ::128].max(axis=0).astype(np.int64)
    DN = (np.maximum(DN, 1) + 3) // 4 * 4      # mult of 4 for add-halving
    poff = np.zeros(NWIN + 1, np.int64)
    np.cumsum(DN * D, out=poff[1:])
    PT = int(poff[-1])

    runs = []
    k = 0
    while k < NWIN:
        k2 = k
        while k2 < NWIN and DN[k2] == DN[k]:
            k2 += 1
        dn = int(DN[k])
        max_nw = max(1, B_MAX_FREE // (D * dn))
        while k < k2:
            k1 = min(k2, k + max_nw)
            runs.append((dn, k, k1, int(poff[k])))
            k = k1

    prepsA = [_prep_a(u_s[m], w_s[m], x_bf, qof, tof, qcolofs, TCP)
              for m in range(N_CORES)]
    return dict(x_bf=x_bf, u_s=u_s, v_s=v_s, w_s=w_s, W8=W8, TCP=TCP,
                pieces=pieces, degs=degs, perms=perms, DN=DN, poff=poff,
                PT=PT, runs=runs, prepsA=prepsA)


def make_in_maps_b(P, resA):
    """Permute launch A's messages into the B-side window layout."""
    in_maps_b = []
    for m in range(N_CORES):
        msgsA = resA.results[m]["MSG"]                # [128, TCP] bf16
        _, col, j = P["prepsA"][m]
        vecs = msgsA[(j * D)[:, None] + np.arange(D)[None, :], col[:, None]]

        vs = P["v_s"][m]
        ordB = np.argsort(vs, kind="stable")
        vB = vs[ordB]
        deg = P["degs"][m]
        startsB = np.zeros(VSH + 1, np.int64)
        np.cumsum(deg, out=startsB[1:])
        s_of = np.arange(vB.shape[0]) - startsB[vB]   # slot within node
        rank_of_node = np.empty(VSH, np.int64)
        rank_of_node[P["perms"][m]] = np.arange(VSH)
        r = rank_of_node[vB]
        kw = r // 128
        p = r % 128
        dnk = P["DN"][kw]
        base = P["poff"][kw] + s_of
        flat = np.zeros((128, P["PT"]), BF16)
        flat[p[:, None], base[:, None] + np.arange(D)[None, :] * dnk[:, None]] = vecs[ordB]
        in_maps_b.append({"MSGB": flat})
    return in_maps_b


def kernel(x, W, u, v, widx):
    P = prep_all(x, W, u, v, widx)

    # ---- launch A: per-edge transform ---------------------------------
    ncA = _build_kernel_a(P["TCP"], P["pieces"])
    in_maps_a = [{"XU": p[0], "W8": P["W8"]} for p in P["prepsA"]]
    LAST_EXEC_NS.clear()
    resA = run_bass_kernel_spmd(ncA, in_maps_a, list(range(N_CORES)), trace=TRACE)
    if TRACE:
        LAST_EXEC_NS.append(resA.exec_time_ns)

    # ---- host: permute messages widx-order -> v-order -----------------
    in_maps_b = make_in_maps_b(P, resA)

    # ---- launch B: segment-sum + ReLU ---------------------------------
    ncB = _build_kernel_b(P["PT"], P["runs"])
    resB = run_bass_kernel_spmd(ncB, in_maps_b, list(range(N_CORES)), trace=TRACE)
    if TRACE:
        LAST_EXEC_NS.append(resB.exec_time_ns)

    out = np.empty((N_NODES, D), np.float32)
    for m in range(N_CORES):
        outP = resB.results[m]["OUTP"]                # [128, NWIN*D]
        byrank = outP.reshape(128, NWIN, D).transpose(1, 0, 2).reshape(NWIN * 128, D)
        out[m * VSH + P["perms"][m]] = byrank[:VSH].astype(np.float32)
    return out
